# revision 5
# baseline (speedup 1.0000x reference)
"""Multi-head causal attention (B=4, T=2048, D=512, H=8) on 8 TRN2 NeuronCores.

Sharding: core c handles batch b = c//2 and head-group hg = c%2 (4 heads,
256 output dims).  No collectives needed — 8 fully independent problems.

Per-core algorithm (all matmul inputs bf16, PSUM accumulation f32):
  - host passes x^T (D,T) and W^T slices (D, 256) in bf16, plus causal masks
  - Q^T,K^T projections:  qT[dh2,T] = (W2h xT), two heads stacked per tile
  - V projection into "augmented V" tiles [k-tile 128, 65] (ones column
    appended -> matmul also produces the softmax denominator row)
  - flash-style: S^T[k,q] = K^T.T @ Q^T per (k-tile, q-block), exp via ACT
    (scale=1/8 folded in, no max subtraction: |scores| <~ 4), causal mask
    multiply on diagonal tiles, O^T accumulation in PSUM over k-tiles
  - O^T -> PE-transpose -> divide by denominator -> natural [T,256] -> DMA out
"""

import numpy as np
import ml_dtypes

T = 2048
D = 512
HG = 4  # heads per core
DH = 64
OUTW = HG * DH  # 256
QB = 512  # q block (columns of S^T tiles)
NQB = T // QB  # 4
NKT = T // 128  # 16 k-tiles
N_CORES = 8

_CACHE = {}


def _build_nc():
    import concourse.bacc as bacc
    import concourse.tile as tile
    import concourse.mybir as mybir
    from concourse.masks import make_identity
    from contextlib import ExitStack

    fp32 = mybir.dt.float32
    bf16 = mybir.dt.bfloat16

    nc = bacc.Bacc(None, target_bir_lowering=False)

    xt_d = nc.declare_dram_parameter("xt", [D, T], bf16, isOutput=False)
    wqt_d = nc.declare_dram_parameter("wqt", [D, OUTW], bf16, isOutput=False)
    wkt_d = nc.declare_dram_parameter("wkt", [D, OUTW], bf16, isOutput=False)
    wvt_d = nc.declare_dram_parameter("wvt", [D, OUTW], bf16, isOutput=False)
    cmask_d = nc.declare_dram_parameter("cmask", [128, T], bf16, isOutput=False)
    out_d = nc.declare_dram_parameter("out", [T, OUTW], fp32, isOutput=True)

    with tile.TileContext(nc) as tc, ExitStack() as ctx:
        const = ctx.enter_context(tc.tile_pool(name="const", bufs=1))
        ps_s = ctx.enter_context(tc.tile_pool(name="ps_s", bufs=2, space="PSUM"))
        ps_o = ctx.enter_context(tc.tile_pool(name="ps_o", bufs=2, space="PSUM"))
        ps_t = ctx.enter_context(tc.tile_pool(name="ps_t", bufs=2, space="PSUM"))
        pt_pool = ctx.enter_context(tc.tile_pool(name="pt", bufs=4))
        osb_pool = ctx.enter_context(tc.tile_pool(name="osb", bufs=2))
        rec_pool = ctx.enter_context(tc.tile_pool(name="rec", bufs=4))

        ident = const.tile([128, 128], fp32, name="ident")
        make_identity(nc, ident[:])

        mask_sb = const.tile([128, T], bf16, name="mask_sb")
        nc.sync.dma_start(out=mask_sb[:], in_=cmask_d[:])

        # x^T: 4 partition-chunks of [128, T]
        xT = []
        for c in range(4):
            t = const.tile([128, T], bf16, tag=f"xT{c}", name=f"xT{c}")
            nc.sync.dma_start(out=t[:], in_=xt_d[c * 128:(c + 1) * 128, :])
            xT.append(t)

        # weight slices W^T chunk c: [128, OUTW]
        def load_wt(dram, name):
            ts = []
            for c in range(4):
                t = const.tile([128, OUTW], bf16, tag=f"{name}{c}", name=f"{name}{c}")
                nc.sync.dma_start(out=t[:], in_=dram[c * 128:(c + 1) * 128, :])
                ts.append(t)
            return ts

        wqT = load_wt(wqt_d, "wqT")
        wkT = load_wt(wkt_d, "wkT")
        wvT = load_wt(wvt_d, "wvT")

        # ---- projections ----
        # Q^T / K^T: per 2-head group g, tile [128, T] (partitions: head 2g on
        # 0:64, head 2g+1 on 64:128)
        qT = [const.tile([128, T], bf16, tag=f"qT{g}", name=f"qT{g}") for g in range(2)]
        kT = [const.tile([128, T], bf16, tag=f"kT{g}", name=f"kT{g}") for g in range(2)]
        for dst, wt in ((qT, wqT), (kT, wkT)):
            for g in range(2):
                for qb4 in range(4):
                    ps = ps_s.tile([128, 1024], fp32, tag="ps", name="ps")
                    for c in range(4):
                        nc.tensor.matmul(
                            ps[:, 0:QB],
                            wt[c][:, g * 128:(g + 1) * 128],
                            xT[c][:, qb4 * QB:(qb4 + 1) * QB],
                            start=(c == 0),
                            stop=(c == 3),
                        )
                    nc.any.tensor_copy(
                        dst[g][:, qb4 * QB:(qb4 + 1) * QB], ps[:, 0:QB]
                    )

        # V augmented: per head h, [128, NKT, 65] (k within tile, k-tile, dh+1)
        vaug = [const.tile([128, NKT, 65], bf16, tag=f"vaug{h}", name=f"vaug{h}") for h in range(HG)]
        for h in range(HG):
            nc.vector.memset(vaug[h][:, :, 64:65], 1.0)
        for tt in range(NKT):
            ps = ps_s.tile([128, 1024], fp32, tag="ps", name="ps")
            for c in range(4):
                nc.tensor.matmul(
                    ps[:, 0:OUTW],
                    xT[c][:, tt * 128:(tt + 1) * 128],
                    wvT[c][:, 0:OUTW],
                    start=(c == 0),
                    stop=(c == 3),
                )
            for h in range(HG):
                nc.any.tensor_copy(
                    vaug[h][:, tt, 0:64], ps[:, h * 64:(h + 1) * 64]
                )

        # output staging: [128, 16, OUTW] f32 (q-tile-major natural layout)
        out_sb = const.tile([128, NQB * 4, OUTW], fp32, tag="out_sb", name="out_sb")

        # ---- attention ----
        for h in range(HG):
            g, po = h // 2, 64 * (h % 2)
            for qb in range(NQB):
                nkt = (qb + 1) * 4
                ot = ps_o.tile([128, QB], fp32, tag="ot", name="ot")
                for kt0 in range(0, nkt, 2):
                    # two k-tiles share one [128,1024] PSUM tile -> one exp
                    st = ps_s.tile([128, 1024], fp32, tag="ps", name="st")
                    for kk in range(2):
                        kt = kt0 + kk
                        nc.tensor.matmul(
                            st[:, kk * QB:(kk + 1) * QB],
                            kT[g][po:po + 64, kt * 128:(kt + 1) * 128],
                            qT[g][po:po + 64, qb * QB:(qb + 1) * QB],
                            start=True,
                            stop=True,
                        )
                    pt = pt_pool.tile([128, 1024], bf16, tag="pt", name="pt")
                    nc.scalar.activation(
                        pt[:], st[:],
                        func=mybir.ActivationFunctionType.Exp, scale=0.125,
                    )
                    for kk in range(2):
                        kt = kt0 + kk
                        j = kt - qb * 4
                        if j >= 0:
                            nc.vector.tensor_mul(
                                pt[:, kk * QB:(kk + 1) * QB],
                                pt[:, kk * QB:(kk + 1) * QB],
                                mask_sb[:, j * QB:(j + 1) * QB],
                            )
                        nc.tensor.matmul(
                            ot[0:65, :],
                            vaug[h][:, kt, :],
                            pt[:, kk * QB:(kk + 1) * QB],
                            start=(kt == 0),
                            stop=(kt == nkt - 1),
                        )
                # normalize + transpose to natural layout
                osb = osb_pool.tile([65, QB], fp32, tag="osb", name="osb")
                nc.vector.tensor_copy(osb[:], ot[0:65, :])
                for j4 in range(4):
                    tp = ps_t.tile([128, 128], fp32, tag="tp", name="tp")
                    nc.tensor.transpose(
                        tp[:, 0:65],
                        osb[:, j4 * 128:(j4 + 1) * 128],
                        ident[0:65, 0:65],
                    )
                    rec = rec_pool.tile([128, 1], fp32, tag="rec", name="rec")
                    nc.vector.reciprocal(rec[:], tp[:, 64:65])
                    qt_idx = qb * 4 + j4
                    nc.vector.tensor_scalar_mul(
                        out_sb[:, qt_idx, h * 64:(h + 1) * 64],
                        tp[:, 0:64],
                        rec[:],
                    )

        for tt in range(NQB * 4):
            nc.sync.dma_start(
                out=out_d[tt * 128:(tt + 1) * 128, :], in_=out_sb[:, tt, :]
            )

    nc.finalize()
    return nc


def _get_nc():
    if "nc" not in _CACHE:
        _CACHE["nc"] = _build_nc()
    return _CACHE["nc"]


def _make_cmask():
    # cmask[p, j*512 + f] = 1.0 iff j*128 + p <= f   (j = diag k-tile offset)
    p = np.arange(128)[:, None]
    f = np.arange(QB)[None, :]
    blocks = [(j * 128 + p <= f).astype(np.float32) for j in range(4)]
    return np.concatenate(blocks, axis=1).astype(ml_dtypes.bfloat16)


def kernel(x, Wq, Wk, Wv):
    from concourse.bass_utils import run_bass_kernel_spmd

    nc = _get_nc()
    bf = ml_dtypes.bfloat16
    cmask = _make_cmask()

    in_maps = []
    for c in range(N_CORES):
        b, hg = c // 2, c % 2
        r0 = hg * OUTW
        in_maps.append({
            "xt": np.ascontiguousarray(x[b].T).astype(bf),
            "wqt": np.ascontiguousarray(Wq[r0:r0 + OUTW].T).astype(bf),
            "wkt": np.ascontiguousarray(Wk[r0:r0 + OUTW].T).astype(bf),
            "wvt": np.ascontiguousarray(Wv[r0:r0 + OUTW].T).astype(bf),
            "cmask": cmask,
        })

    res = run_bass_kernel_spmd(nc, in_maps, core_ids=list(range(N_CORES)))

    B = x.shape[0]
    out = np.empty((B, T, D), dtype=np.float32)
    for c in range(N_CORES):
        b, hg = c // 2, c % 2
        out[b, :, hg * OUTW:(hg + 1) * OUTW] = res.results[c]["out"]
    return out
